# revision 4
# baseline (speedup 1.0000x reference)
"""Trainium2 Bass kernel for nn_GatedAttention.

Data-parallel over batch: N=8 examples -> 8 NeuronCores, one example per core.
Per core (l=2048 query rows, m=512 memory rows, d=1024 in-dim, h=1024 hid):

  input_dot^T[h,l]  = relu(W1^T @ x^T + b1)         (MM1; x transposed on PE)
  memory_dot^T[h,m] = relu(Wm^T @ mem^T + bm)       (MM2)
  att[l,m]          = input_dot @ memory_dot^T      (MM3, raw scores)
  w[l,m]            = softmax(att/32, masked)       (DVE/ACT, exp-normalize)
  output_one^T[d,l] = mem^T @ w^T                   (MM4; w transposed on PE)
  pre[l,h]          = [x, output_one] @ W2 + b2     (MM5)
  out[l,h]          = sigmoid(pre) * tanh(pre)

All matmul operands are bf16 (fp32 PSUM accumulation); softmax and the
gated epilogue run in fp32.
"""

from contextlib import ExitStack

import numpy as np

import concourse.bacc as bacc
import concourse.bass as bass
import concourse.mybir as mybir
import concourse.tile as tile
from concourse import bass_utils
from concourse.masks import make_identity

LD, LM, D, H = 2048, 512, 1024, 1024
CT = 16  # concat c-tiles (2048 / 128)
DT, HT, MT = D // 128, H // 128, LM // 128
NCHUNK, LC = 4, 512  # l processed in 4 chunks of 512
LT_PER_CHUNK = LC // 128
N_CORES = 8

FP32 = mybir.dt.float32
BF16 = mybir.dt.bfloat16
AF = mybir.ActivationFunctionType
ALU = mybir.AluOpType
AX = mybir.AxisListType
INV_DOT_SCALE = 1.0 / 32.0  # 1/sqrt(D)


def _bcast_rows(ap, parts, free):
    """AP reading a 1-D DRAM tensor broadcast across `parts` partitions."""
    return bass.AP(tensor=ap.tensor, offset=ap.offset, ap=[[0, parts], [1, free]])


def _col_tile(ap, parts, free):
    """AP viewing a 1-D DRAM tensor [parts*free] as [parts, free] column tiles."""
    return bass.AP(tensor=ap.tensor, offset=ap.offset, ap=[[1, parts], [parts, free]])


def _emit(nc, tc, out, x, mem, mask, W1, b1, Wm, bm, W2, b2):
    with ExitStack() as ctx:
        const = ctx.enter_context(tc.tile_pool(name="const", bufs=1))
        big = ctx.enter_context(tc.tile_pool(name="big", bufs=1))
        stage = ctx.enter_context(tc.tile_pool(name="stage", bufs=6))
        soft = ctx.enter_context(tc.tile_pool(name="soft", bufs=2))
        epi = ctx.enter_context(tc.tile_pool(name="epi", bufs=2))
        ps_mm = ctx.enter_context(tc.tile_pool(name="ps_mm", bufs=5, space="PSUM"))
        ps_tp = ctx.enter_context(tc.tile_pool(name="ps_tp", bufs=2, space="PSUM"))

        # ---- constants -------------------------------------------------
        idn = const.tile([128, 128], BF16)
        make_identity(nc, idn)
        idn_f32 = const.tile([128, 128], FP32)
        make_identity(nc, idn_f32)
        b1sb = const.tile([128, HT], FP32)
        nc.gpsimd.dma_start(out=b1sb, in_=_col_tile(b1, 128, HT))
        bmsb = const.tile([128, HT], FP32)
        nc.gpsimd.dma_start(out=bmsb, in_=_col_tile(bm, 128, HT))
        b2row = const.tile([128, H], FP32)
        nc.gpsimd.dma_start(out=b2row, in_=_bcast_rows(b2, 128, H))
        maskrow = const.tile([128, LM], FP32)
        nc.gpsimd.dma_start(out=maskrow, in_=_bcast_rows(mask, 128, LM))

        # ---- persistent bf16 tensors ----------------------------------
        W1sb = big.tile([128, DT, H], BF16)    # [d_part, d_tile, h]
        W2sb = big.tile([128, CT, H], BF16)    # [c_part, c_tile, h]
        mem_bf = big.tile([128, MT, D], BF16)  # [m_part, m_tile, d]
        xT = big.tile([128, DT, LD], BF16)     # x^T: [d_part, d_tile, l]
        mdT = big.tile([128, HT, LM], BF16)    # memory_dot^T: [h_part, h_tile, m]

        def load_cast(dst3, src, n_tiles, width, label):
            for t in range(n_tiles):
                ws = stage.tile([128, width], FP32, tag="stg", name=f"ld_{label}_{t}")
                nc.sync.dma_start(out=ws, in_=src[t * 128 : (t + 1) * 128, :])
                nc.any.tensor_copy(out=dst3[:, t, :], in_=ws)

        # DMA x rows for one chunk; return staged fp32 tiles
        def dma_x_chunk(j):
            tiles = []
            for lt in range(LT_PER_CHUNK):
                ltg = j * LT_PER_CHUNK + lt
                xs = stage.tile([128, D], FP32, tag="stg", name=f"xs_{ltg}")
                nc.sync.dma_start(out=xs, in_=x[ltg * 128 : (ltg + 1) * 128, :])
                tiles.append(xs)
            return tiles

        # PE-transpose staged fp32 rows into xT (bf16 via copy-out cast)
        def transpose_x_chunk(j, tiles):
            for lt in range(LT_PER_CHUNK):
                ltg = j * LT_PER_CHUNK + lt
                xs = tiles[lt]
                for half in range(2):
                    px = ps_tp.tile(
                        [128, 512], FP32, tag="tp", name=f"px_{ltg}_{half}"
                    )
                    for q in range(4):
                        dt = half * 4 + q
                        nc.tensor.transpose(
                            px[:, q * 128 : (q + 1) * 128],
                            xs[:, dt * 128 : (dt + 1) * 128],
                            idn_f32,
                        )
                    nc.vector.tensor_copy(
                        out=xT[:, half * 4 : half * 4 + 4, ltg * 128 : (ltg + 1) * 128],
                        in_=px.rearrange("p (t q) -> p t q", q=128),
                    )

        # ---- load order: x0, W1, mem, (x1 inside chunk 0), Wm, W2 -----
        x_tiles = dma_x_chunk(0)
        load_cast(W1sb, W1, DT, H, "W1")

        mem_tiles = []
        for mt in range(MT):
            ms = stage.tile([128, D], FP32, tag="stg", name=f"ms_{mt}")
            nc.sync.dma_start(out=ms, in_=mem[mt * 128 : (mt + 1) * 128, :])
            mem_tiles.append(ms)
            nc.any.tensor_copy(out=mem_bf[:, mt, :], in_=ms)

        transpose_x_chunk(0, x_tiles)

        # ---- pre pool: Wm + memT (freed after MM2) ---------------------
        with tc.tile_pool(name="pre", bufs=1) as pre:
            Wmsb = pre.tile([128, DT, H], BF16)
            memT = pre.tile([128, DT, LM], BF16)  # mem^T: [d_part, d_tile, m]

            # transpose mem (fp32 staged -> bf16 memT)
            for mt in range(MT):
                ms = mem_tiles[mt]
                for half in range(2):
                    pm = ps_tp.tile([128, 512], FP32, tag="tp", name=f"pm_{mt}_{half}")
                    for q in range(4):
                        dt = half * 4 + q
                        nc.tensor.transpose(
                            pm[:, q * 128 : (q + 1) * 128],
                            ms[:, dt * 128 : (dt + 1) * 128],
                            idn_f32,
                        )
                    nc.vector.tensor_copy(
                        out=memT[:, half * 4 : half * 4 + 4, mt * 128 : (mt + 1) * 128],
                        in_=pm.rearrange("p (t q) -> p t q", q=128),
                    )

            load_cast(Wmsb, Wm, DT, H, "Wm")

            # MM2: memory_dot^T[h,m] = relu(Wm^T @ mem^T + bm)
            for ht in range(HT):
                ps = ps_mm.tile([128, LM], FP32, tag="mm", name=f"mm2_{ht}")
                for dt in range(DT):
                    nc.tensor.matmul(
                        ps,
                        Wmsb[:, dt, ht * 128 : (ht + 1) * 128],
                        memT[:, dt, :],
                        start=(dt == 0),
                        stop=(dt == DT - 1),
                    )
                nc.scalar.activation(
                    out=mdT[:, ht, :], in_=ps, func=AF.Relu, bias=bmsb[:, ht : ht + 1]
                )

        # ---- chunk pools (address space reused from `pre`) -------------
        chunk = ctx.enter_context(tc.tile_pool(name="chunk", bufs=2))

        for j in range(NCHUNK):
            idT_j = chunk.tile([128, HT, LC], BF16, tag="idT", name=f"idT_{j}")
            wT_j = chunk.tile([128, MT, LC], BF16, tag="wT", name=f"wT_{j}")
            ooT_j = chunk.tile([128, DT, LC], BF16, tag="ooT", name=f"ooT_{j}")

            if j + 1 < NCHUNK:
                x_tiles = dma_x_chunk(j + 1)

            # MM1: input_dot^T[h, l-chunk] = relu(W1^T @ x^T + b1)
            for ht in range(HT):
                ps = ps_mm.tile([128, LC], FP32, tag="mm", name=f"mm1_{j}_{ht}")
                for dt in range(DT):
                    nc.tensor.matmul(
                        ps,
                        W1sb[:, dt, ht * 128 : (ht + 1) * 128],
                        xT[:, dt, j * LC : (j + 1) * LC],
                        start=(dt == 0),
                        stop=(dt == DT - 1),
                    )
                nc.scalar.activation(
                    out=idT_j[:, ht, :], in_=ps, func=AF.Relu, bias=b1sb[:, ht : ht + 1]
                )

            # next chunk's transposes slot in after MM1 on the PE stream
            if j + 1 < NCHUNK:
                transpose_x_chunk(j + 1, x_tiles)
            if j == 0:
                load_cast(W2sb, W2, CT, H, "W2")

            # MM3 + softmax + w-transpose, per l-tile
            for lt in range(LT_PER_CHUNK):
                ps = ps_mm.tile([128, LM], FP32, tag="mm", name=f"mm3_{j}_{lt}")
                for ht in range(HT):
                    nc.tensor.matmul(
                        ps,
                        idT_j[:, ht, lt * 128 : (lt + 1) * 128],
                        mdT[:, ht, :],
                        start=(ht == 0),
                        stop=(ht == HT - 1),
                    )
                negmax = soft.tile([128, 1], FP32, tag="negmax", name=f"ngm_{j}_{lt}")
                nc.vector.tensor_reduce(
                    out=negmax, in_=ps, axis=AX.X, op=ALU.max, negate=True
                )
                negmax32 = soft.tile([128, 1], FP32, tag="negmax32", name=f"ngs_{j}_{lt}")
                nc.vector.tensor_scalar_mul(negmax32, negmax, INV_DOT_SCALE)
                # E = exp(att/32 - max/32)
                e_f = soft.tile([128, LM], FP32, tag="e_f", name=f"ef_{j}_{lt}")
                nc.scalar.activation(
                    out=e_f, in_=ps, func=AF.Exp, bias=negmax32, scale=INV_DOT_SCALE
                )
                # multiplicative mask (exact for 0/1 masks), fused row-sum Z
                zsum = soft.tile([128, 1], FP32, tag="zsum", name=f"z_{j}_{lt}")
                e_m = soft.tile([128, LM], FP32, tag="e_m", name=f"em_{j}_{lt}")
                nc.vector.scalar_tensor_tensor(
                    out=e_m,
                    in0=e_f,
                    scalar=1.0,
                    in1=maskrow,
                    op0=ALU.mult,
                    op1=ALU.mult,
                    accum_out=zsum,
                )
                rz = soft.tile([128, 1], FP32, tag="rz", name=f"rz_{j}_{lt}")
                nc.vector.reciprocal(out=rz, in_=zsum)
                w_bf = soft.tile([128, LM], BF16, tag="w_bf", name=f"wb_{j}_{lt}")
                nc.vector.tensor_scalar_mul(w_bf, e_m, rz)
                # transpose w -> wT (bf16 PE transpose)
                pw = ps_tp.tile([128, LM], BF16, tag="tp", name=f"pw_{j}_{lt}")
                for mt in range(MT):
                    nc.tensor.transpose(
                        pw[:, mt * 128 : (mt + 1) * 128],
                        w_bf[:, mt * 128 : (mt + 1) * 128],
                        idn,
                    )
                nc.vector.tensor_copy(
                    out=wT_j[:, :, lt * 128 : (lt + 1) * 128],
                    in_=pw.rearrange("p (t q) -> p t q", q=128),
                )

            # MM4: output_one^T[d, l-chunk] = mem^T @ w^T
            for dt in range(DT):
                ps = ps_mm.tile([128, LC], FP32, tag="mm", name=f"mm4_{j}_{dt}")
                for mt in range(MT):
                    nc.tensor.matmul(
                        ps,
                        mem_bf[:, mt, dt * 128 : (dt + 1) * 128],
                        wT_j[:, mt, :],
                        start=(mt == 0),
                        stop=(mt == MT - 1),
                    )
                nc.any.tensor_copy(out=ooT_j[:, dt, :], in_=ps)

            # MM5: pre[l,h] = [x, output_one] @ W2 + b2 ; out = sig*tanh
            for lt in range(LT_PER_CHUNK):
                ltg = j * LT_PER_CHUNK + lt
                for hc in range(2):
                    ps = ps_mm.tile([128, 512], FP32, tag="mm", name=f"mm5_{ltg}_{hc}")
                    for ct in range(CT):
                        if ct < DT:
                            lhsT = xT[:, ct, ltg * 128 : (ltg + 1) * 128]
                        else:
                            lhsT = ooT_j[:, ct - DT, lt * 128 : (lt + 1) * 128]
                        nc.tensor.matmul(
                            ps,
                            lhsT,
                            W2sb[:, ct, hc * 512 : (hc + 1) * 512],
                            start=(ct == 0),
                            stop=(ct == CT - 1),
                        )
                    s_in = epi.tile([128, 512], FP32, tag="s_in", name=f"si_{ltg}_{hc}")
                    nc.vector.scalar_tensor_tensor(
                        out=s_in,
                        in0=ps,
                        scalar=1.0,
                        in1=b2row[:, hc * 512 : (hc + 1) * 512],
                        op0=ALU.mult,
                        op1=ALU.add,
                    )
                    sg = epi.tile([128, 512], FP32, tag="sg", name=f"sg_{ltg}_{hc}")
                    nc.scalar.activation(out=sg, in_=s_in, func=AF.Sigmoid)
                    th = epi.tile([128, 512], FP32, tag="th", name=f"th_{ltg}_{hc}")
                    nc.scalar.activation(out=th, in_=s_in, func=AF.Tanh)
                    outt = epi.tile([128, 512], FP32, tag="outt", name=f"ot_{ltg}_{hc}")
                    nc.vector.tensor_mul(outt, sg, th)
                    nc.sync.dma_start(
                        out=out[ltg * 128 : (ltg + 1) * 128, hc * 512 : (hc + 1) * 512],
                        in_=outt,
                    )


_NC_CACHE = {}


def _build_nc():
    if "nc" in _NC_CACHE:
        return _NC_CACHE["nc"]
    nc = bacc.Bacc("TRN2", target_bir_lowering=False, debug=False)
    aps = {}
    for name, shape in [
        ("x", (LD, D)),
        ("mem", (LM, D)),
        ("mask", (LM,)),
        ("W1", (D, H)),
        ("b1", (H,)),
        ("Wm", (D, H)),
        ("bm", (H,)),
        ("W2", (2 * D, H)),
        ("b2", (H,)),
    ]:
        aps[name] = nc.dram_tensor(name, shape, FP32, kind="ExternalInput").ap()
    out = nc.dram_tensor("out", (LD, H), FP32, kind="ExternalOutput").ap()

    with tile.TileContext(nc) as tc:
        _emit(
            nc, tc, out,
            aps["x"], aps["mem"], aps["mask"],
            aps["W1"], aps["b1"], aps["Wm"], aps["bm"], aps["W2"], aps["b2"],
        )
    nc.compile()
    _NC_CACHE["nc"] = nc
    return nc


def kernel(x, mem, mask, W1, b1, Wm, bm, W2, b2, _trace=False):
    x = np.ascontiguousarray(np.asarray(x, dtype=np.float32))
    mem = np.ascontiguousarray(np.asarray(mem, dtype=np.float32))
    mask = np.ascontiguousarray(np.asarray(mask, dtype=np.float32))
    shared = {
        "W1": np.ascontiguousarray(np.asarray(W1, np.float32)),
        "b1": np.ascontiguousarray(np.asarray(b1, np.float32)),
        "Wm": np.ascontiguousarray(np.asarray(Wm, np.float32)),
        "bm": np.ascontiguousarray(np.asarray(bm, np.float32)),
        "W2": np.ascontiguousarray(np.asarray(W2, np.float32)),
        "b2": np.ascontiguousarray(np.asarray(b2, np.float32)),
    }
    nc = _build_nc()
    in_maps = [
        {"x": x[c], "mem": mem[c], "mask": mask[c], **shared} for c in range(N_CORES)
    ]
    res = bass_utils.run_bass_kernel_spmd(
        nc, in_maps, core_ids=list(range(N_CORES)), trace=_trace
    )
    output = np.stack([res.results[c]["out"] for c in range(N_CORES)])
    if _trace:
        kernel._last_result = res
    return output, mem


# revision 10
# speedup vs baseline: 1.0253x; 1.0253x over previous
"""Trainium2 Bass kernel for nn_GatedAttention.

Data-parallel over batch: N=8 examples -> 8 NeuronCores, one example per core.
Per core (l=2048 query rows, m=512 memory rows, d=1024 in-dim, h=1024 hid):

  input_dot^T[h,l]  = relu(W1^T @ x^T + b1)         (MM1; x transposed on PE)
  memory_dot^T[h,m] = relu(Wm^T @ mem^T + bm)       (MM2)
  att[l,m]          = input_dot @ memory_dot^T      (MM3, raw scores)
  w[l,m]            = softmax(att/32, masked)       (DVE/ACT, exp-normalize)
  output_one^T[d,l] = mem^T @ w^T                   (MM4; w transposed on PE)
  pre[l,h]          = [x, output_one] @ W2 + b2     (MM5)
  out[l,h]          = sigmoid(pre) * tanh(pre)

All matmul operands are bf16 (fp32 PSUM accumulation); softmax and the
gated epilogue run in fp32.
"""

from contextlib import ExitStack

import numpy as np

import concourse.bacc as bacc
import concourse.bass as bass
import concourse.mybir as mybir
import concourse.tile as tile
from concourse import bass_utils
from concourse.masks import make_identity

LD, LM, D, H = 2048, 512, 1024, 1024
CT = 16  # concat c-tiles (2048 / 128)
DT, HT, MT = D // 128, H // 128, LM // 128
NCHUNK, LC = 4, 512  # l processed in 4 chunks of 512
LT_PER_CHUNK = LC // 128
N_CORES = 8

FP32 = mybir.dt.float32
BF16 = mybir.dt.bfloat16
AF = mybir.ActivationFunctionType
ALU = mybir.AluOpType
AX = mybir.AxisListType
INV_DOT_SCALE = 1.0 / 32.0  # 1/sqrt(D)


def _bcast_rows(ap, parts, free):
    """AP reading a 1-D DRAM tensor broadcast across `parts` partitions."""
    return bass.AP(tensor=ap.tensor, offset=ap.offset, ap=[[0, parts], [1, free]])


def _col_tile(ap, parts, free):
    """AP viewing a 1-D DRAM tensor [parts*free] as [parts, free] column tiles."""
    return bass.AP(tensor=ap.tensor, offset=ap.offset, ap=[[1, parts], [parts, free]])


def _emit(nc, tc, out, x, mem, mask, W1, b1, Wm, bm, W2, b2):
    with ExitStack() as ctx:
        const = ctx.enter_context(tc.tile_pool(name="const", bufs=1))
        big = ctx.enter_context(tc.tile_pool(name="big", bufs=1))
        stage = ctx.enter_context(tc.tile_pool(name="stage", bufs=4))
        soft = ctx.enter_context(tc.tile_pool(name="soft", bufs=2))
        epi = ctx.enter_context(tc.tile_pool(name="epi", bufs=2))
        ps_mm = ctx.enter_context(tc.tile_pool(name="ps_mm", bufs=5, space="PSUM"))
        ps_tp = ctx.enter_context(tc.tile_pool(name="ps_tp", bufs=2, space="PSUM"))

        # ---- constants -------------------------------------------------
        idn = const.tile([128, 128], BF16)
        make_identity(nc, idn)
        idn_f32 = const.tile([128, 128], FP32)
        make_identity(nc, idn_f32)
        b1sb = const.tile([128, HT], FP32)
        nc.gpsimd.dma_start(out=b1sb, in_=_col_tile(b1, 128, HT))
        bmsb = const.tile([128, HT], FP32)
        nc.gpsimd.dma_start(out=bmsb, in_=_col_tile(bm, 128, HT))
        b2row = const.tile([128, H], FP32)
        nc.gpsimd.dma_start(out=b2row, in_=_bcast_rows(b2, 128, H))
        maskrow = const.tile([128, LM], FP32)
        nc.gpsimd.dma_start(out=maskrow, in_=_bcast_rows(mask, 128, LM))

        # ---- persistent bf16 tensors ----------------------------------
        W1sb = big.tile([128, DT, H], BF16)    # [d_part, d_tile, h]
        W2sb = big.tile([128, CT, H], BF16)    # [c_part, c_tile, h]
        mem_bf = big.tile([128, MT, D], BF16)  # [m_part, m_tile, d]
        xT = big.tile([128, DT, LD], BF16)     # x^T: [d_part, d_tile, l]
        mdT = big.tile([128, HT, LM], BF16)    # memory_dot^T: [h_part, h_tile, m]

        def load_cast(dst3, src, n_tiles, width, label):
            for t in range(n_tiles):
                ws = stage.tile([128, width], FP32, tag="stg", name=f"ld_{label}_{t}")
                nc.sync.dma_start(out=ws, in_=src[t * 128 : (t + 1) * 128, :])
                nc.any.tensor_copy(out=dst3[:, t, :], in_=ws)

        # DMA x rows for one chunk; return staged fp32 tiles
        def dma_x_chunk(j):
            tiles = []
            for lt in range(LT_PER_CHUNK):
                ltg = j * LT_PER_CHUNK + lt
                xs = stage.tile([128, D], FP32, tag="stg", name=f"xs_{ltg}")
                nc.sync.dma_start(out=xs, in_=x[ltg * 128 : (ltg + 1) * 128, :])
                tiles.append(xs)
            return tiles

        # PE-transpose staged fp32 rows into xT (bf16 via copy-out cast)
        def transpose_x_chunk(j, tiles):
            for lt in range(LT_PER_CHUNK):
                ltg = j * LT_PER_CHUNK + lt
                xs = tiles[lt]
                for half in range(2):
                    px = ps_tp.tile(
                        [128, 512], FP32, tag="tp", name=f"px_{ltg}_{half}"
                    )
                    for q in range(4):
                        dt = half * 4 + q
                        nc.tensor.transpose(
                            px[:, q * 128 : (q + 1) * 128],
                            xs[:, dt * 128 : (dt + 1) * 128],
                            idn_f32,
                        )
                    nc.any.tensor_copy(
                        out=xT[:, half * 4 : half * 4 + 4, ltg * 128 : (ltg + 1) * 128],
                        in_=px.rearrange("p (t q) -> p t q", q=128),
                    )

        # ---- load order: x0, W1, mem, (x1 inside chunk 0), Wm, W2 -----
        x_tiles = dma_x_chunk(0)
        load_cast(W1sb, W1, DT, H, "W1")

        mem_tiles = []
        for mt in range(MT):
            ms = stage.tile([128, D], FP32, tag="stg", name=f"ms_{mt}")
            nc.sync.dma_start(out=ms, in_=mem[mt * 128 : (mt + 1) * 128, :])
            mem_tiles.append(ms)
            nc.any.tensor_copy(out=mem_bf[:, mt, :], in_=ms)

        transpose_x_chunk(0, x_tiles)

        # MM1: input_dot^T[h, l-chunk] = relu(W1^T @ x^T + b1)
        def mm1_chunk(j):
            for ht in range(HT):
                ps = ps_mm.tile([128, LC], FP32, tag="mm", name=f"mm1_{j}_{ht}")
                for dt in range(DT):
                    nc.tensor.matmul(
                        ps,
                        W1sb[:, dt, ht * 128 : (ht + 1) * 128],
                        xT[:, dt, j * LC : (j + 1) * LC],
                        start=(dt == 0),
                        stop=(dt == DT - 1),
                    )
                nc.scalar.activation(
                    out=idT[:, ht, j * LC : (j + 1) * LC],
                    in_=ps,
                    func=AF.Relu,
                    bias=b1sb[:, ht : ht + 1],
                )

        mm1_chunk(0)

        # ---- pre pool: Wm + memT (freed after MM2) ---------------------
        with tc.tile_pool(name="pre", bufs=1) as pre:
            Wmsb = pre.tile([128, DT, H], BF16)
            memT = pre.tile([128, DT, LM], BF16)  # mem^T: [d_part, d_tile, m]

            # transpose mem (fp32 staged -> bf16 memT)
            for mt in range(MT):
                ms = mem_tiles[mt]
                for half in range(2):
                    pm = ps_tp.tile([128, 512], FP32, tag="tp", name=f"pm_{mt}_{half}")
                    for q in range(4):
                        dt = half * 4 + q
                        nc.tensor.transpose(
                            pm[:, q * 128 : (q + 1) * 128],
                            ms[:, dt * 128 : (dt + 1) * 128],
                            idn_f32,
                        )
                    nc.any.tensor_copy(
                        out=memT[:, half * 4 : half * 4 + 4, mt * 128 : (mt + 1) * 128],
                        in_=pm.rearrange("p (t q) -> p t q", q=128),
                    )

            load_cast(Wmsb, Wm, DT, H, "Wm")

            # MM2: memory_dot^T[h,m] = relu(Wm^T @ mem^T + bm)
            for ht in range(HT):
                ps = ps_mm.tile([128, LM], FP32, tag="mm", name=f"mm2_{ht}")
                for dt in range(DT):
                    nc.tensor.matmul(
                        ps,
                        Wmsb[:, dt, ht * 128 : (ht + 1) * 128],
                        memT[:, dt, :],
                        start=(dt == 0),
                        stop=(dt == DT - 1),
                    )
                nc.scalar.activation(
                    out=mdT[:, ht, :], in_=ps, func=AF.Relu, bias=bmsb[:, ht : ht + 1]
                )

        # ---- chunk pools (address space reused from `pre`) -------------
        chunk = ctx.enter_context(tc.tile_pool(name="chunk", bufs=2))

        for j in range(NCHUNK):
            idT_j = chunk.tile([128, HT, LC], BF16, tag="idT", name=f"idT_{j}")
            wT_j = chunk.tile([128, MT, LC], BF16, tag="wT", name=f"wT_{j}")
            ooT_j = chunk.tile([128, DT, LC], BF16, tag="ooT", name=f"ooT_{j}")

            if j + 1 < NCHUNK:
                x_tiles = dma_x_chunk(j + 1)

            # MM1: input_dot^T[h, l-chunk] = relu(W1^T @ x^T + b1)
            for ht in range(HT):
                ps = ps_mm.tile([128, LC], FP32, tag="mm", name=f"mm1_{j}_{ht}")
                for dt in range(DT):
                    nc.tensor.matmul(
                        ps,
                        W1sb[:, dt, ht * 128 : (ht + 1) * 128],
                        xT[:, dt, j * LC : (j + 1) * LC],
                        start=(dt == 0),
                        stop=(dt == DT - 1),
                    )
                nc.scalar.activation(
                    out=idT_j[:, ht, :], in_=ps, func=AF.Relu, bias=b1sb[:, ht : ht + 1]
                )

            if j == 0:
                load_cast(W2sb, W2, CT, H, "W2")

            # MM3 + softmax, per l-tile
            w_tiles = []
            for lt in range(LT_PER_CHUNK):
                ps = ps_mm.tile([128, LM], FP32, tag="mm", name=f"mm3_{j}_{lt}")
                for ht in range(HT):
                    nc.tensor.matmul(
                        ps,
                        idT[:, ht, j * LC + lt * 128 : j * LC + (lt + 1) * 128],
                        mdT[:, ht, :],
                        start=(ht == 0),
                        stop=(ht == HT - 1),
                    )
                negmax = soft.tile([128, 1], FP32, tag="negmax", name=f"ngm_{j}_{lt}")
                nc.vector.tensor_reduce(
                    out=negmax, in_=ps, axis=AX.X, op=ALU.max, negate=True
                )
                negmax32 = soft.tile([128, 1], FP32, tag="negmax32", name=f"ngs_{j}_{lt}")
                nc.vector.tensor_scalar_mul(negmax32, negmax, INV_DOT_SCALE)
                # E = exp(att/32 - max/32)
                e_f = soft.tile([128, LM], FP32, tag="e_f", name=f"ef_{j}_{lt}")
                nc.scalar.activation(
                    out=e_f, in_=ps, func=AF.Exp, bias=negmax32, scale=INV_DOT_SCALE
                )
                # multiplicative mask (exact for 0/1 masks), fused row-sum Z
                zsum = soft.tile([128, 1], FP32, tag="zsum", name=f"z_{j}_{lt}")
                e_m = soft.tile([128, LM], FP32, tag="e_m", name=f"em_{j}_{lt}")
                nc.vector.scalar_tensor_tensor(
                    out=e_m,
                    in0=e_f,
                    scalar=1.0,
                    in1=maskrow,
                    op0=ALU.mult,
                    op1=ALU.mult,
                    accum_out=zsum,
                )
                rz = soft.tile([128, 1], FP32, tag="rz", name=f"rz_{j}_{lt}")
                nc.vector.reciprocal(out=rz, in_=zsum)
                w_bf = soft.tile(
                    [128, LM], BF16, tag="w_bf", bufs=6, name=f"wb_{j}_{lt}"
                )
                nc.vector.tensor_scalar_mul(w_bf, e_m, rz)
                w_tiles.append(w_bf)

            # next chunk's x transposes fill the softmax latency on PE
            if j + 1 < NCHUNK:
                transpose_x_chunk(j + 1, x_tiles)

            # transpose w -> wT (bf16 PE transpose), hoisted after MM3 loop
            for lt in range(LT_PER_CHUNK):
                w_bf = w_tiles[lt]
                pw = ps_tp.tile([128, LM], BF16, tag="tp", name=f"pw_{j}_{lt}")
                for mt in range(MT):
                    nc.tensor.transpose(
                        pw[:, mt * 128 : (mt + 1) * 128],
                        w_bf[:, mt * 128 : (mt + 1) * 128],
                        idn,
                    )
                nc.any.tensor_copy(
                    out=wT_j[:, :, lt * 128 : (lt + 1) * 128],
                    in_=pw.rearrange("p (t q) -> p t q", q=128),
                )

            # MM4: output_one^T[d, l-chunk] = mem^T @ w^T
            for dt in range(DT):
                ps = ps_mm.tile([128, LC], FP32, tag="mm", name=f"mm4_{j}_{dt}")
                for mt in range(MT):
                    nc.tensor.matmul(
                        ps,
                        mem_bf[:, mt, dt * 128 : (dt + 1) * 128],
                        wT_j[:, mt, :],
                        start=(mt == 0),
                        stop=(mt == MT - 1),
                    )
                nc.any.tensor_copy(out=ooT_j[:, dt, :], in_=ps)

            # MM5: pre[l,h] = [x, output_one] @ W2 + b2 ; out = sig*tanh
            for lt in range(LT_PER_CHUNK):
                ltg = j * LT_PER_CHUNK + lt
                for hc in range(2):
                    ps = ps_mm.tile([128, 512], FP32, tag="mm", name=f"mm5_{ltg}_{hc}")
                    for ct in range(CT):
                        if ct < DT:
                            lhsT = xT[:, ct, ltg * 128 : (ltg + 1) * 128]
                        else:
                            lhsT = ooT_j[:, ct - DT, lt * 128 : (lt + 1) * 128]
                        nc.tensor.matmul(
                            ps,
                            lhsT,
                            W2sb[:, ct, hc * 512 : (hc + 1) * 512],
                            start=(ct == 0),
                            stop=(ct == CT - 1),
                        )
                    s_in = epi.tile([128, 512], FP32, tag="s_in", name=f"si_{ltg}_{hc}")
                    nc.vector.scalar_tensor_tensor(
                        out=s_in,
                        in0=ps,
                        scalar=1.0,
                        in1=b2row[:, hc * 512 : (hc + 1) * 512],
                        op0=ALU.mult,
                        op1=ALU.add,
                    )
                    sg = epi.tile([128, 512], FP32, tag="sg", name=f"sg_{ltg}_{hc}")
                    nc.scalar.activation(out=sg, in_=s_in, func=AF.Sigmoid)
                    th = epi.tile([128, 512], FP32, tag="th", name=f"th_{ltg}_{hc}")
                    nc.scalar.activation(out=th, in_=s_in, func=AF.Tanh)
                    outt = epi.tile([128, 512], FP32, tag="outt", name=f"ot_{ltg}_{hc}")
                    nc.vector.tensor_mul(outt, sg, th)
                    nc.sync.dma_start(
                        out=out[ltg * 128 : (ltg + 1) * 128, hc * 512 : (hc + 1) * 512],
                        in_=outt,
                    )


_NC_CACHE = {}


def _build_nc():
    if "nc" in _NC_CACHE:
        return _NC_CACHE["nc"]
    nc = bacc.Bacc("TRN2", target_bir_lowering=False, debug=False)
    aps = {}
    for name, shape in [
        ("x", (LD, D)),
        ("mem", (LM, D)),
        ("mask", (LM,)),
        ("W1", (D, H)),
        ("b1", (H,)),
        ("Wm", (D, H)),
        ("bm", (H,)),
        ("W2", (2 * D, H)),
        ("b2", (H,)),
    ]:
        aps[name] = nc.dram_tensor(name, shape, FP32, kind="ExternalInput").ap()
    out = nc.dram_tensor("out", (LD, H), FP32, kind="ExternalOutput").ap()

    with tile.TileContext(nc) as tc:
        _emit(
            nc, tc, out,
            aps["x"], aps["mem"], aps["mask"],
            aps["W1"], aps["b1"], aps["Wm"], aps["bm"], aps["W2"], aps["b2"],
        )
    nc.compile()
    _NC_CACHE["nc"] = nc
    return nc


def kernel(x, mem, mask, W1, b1, Wm, bm, W2, b2, _trace=False):
    x = np.ascontiguousarray(np.asarray(x, dtype=np.float32))
    mem = np.ascontiguousarray(np.asarray(mem, dtype=np.float32))
    mask = np.ascontiguousarray(np.asarray(mask, dtype=np.float32))
    shared = {
        "W1": np.ascontiguousarray(np.asarray(W1, np.float32)),
        "b1": np.ascontiguousarray(np.asarray(b1, np.float32)),
        "Wm": np.ascontiguousarray(np.asarray(Wm, np.float32)),
        "bm": np.ascontiguousarray(np.asarray(bm, np.float32)),
        "W2": np.ascontiguousarray(np.asarray(W2, np.float32)),
        "b2": np.ascontiguousarray(np.asarray(b2, np.float32)),
    }
    nc = _build_nc()
    in_maps = [
        {"x": x[c], "mem": mem[c], "mask": mask[c], **shared} for c in range(N_CORES)
    ]
    res = bass_utils.run_bass_kernel_spmd(
        nc, in_maps, core_ids=list(range(N_CORES)), trace=_trace
    )
    output = np.stack([res.results[c]["out"] for c in range(N_CORES)])
    if _trace:
        kernel._last_result = res
    return output, mem
